# revision 53
# baseline (speedup 1.0000x reference)
"""Trainium2 Bass kernel for ChunkedTGnnModel — exact-integer-M fp8 design.

Math (per temporal chunk, flattened to a [128000, 64] slab whose
consecutive 1000-row blocks each see the same dense normalized adjacency
A_hat = D^-1/2 (Adj + I) D^-1/2, [1000 x 1000]):

    out = relu(A_hat @ (block @ W1) + b1)   (layer 2 same)

Core trick: fold the D^-1/2 diagonals OUT of the matmul. The streamed
matrix becomes M = Adj + I whose entries are small INTEGERS ({0,1,2}) —
exactly representable in fp8e4m3. Only the P = X@W operand needs an
error split (P = Ph + Pl), so the dominant A-type matmul needs 2 fp8
DoubleRow products instead of 3 (as an fp8 split of the full A would).

Folding details (lambda = 32 keeps fp8 operands out of the subnormal
range; all scale hops are powers of two except the dinv vectors):
    P1' = (lam * dinv (.) X) @ W1   (host, fused into input packing,
                                     shipped pre-split as fp8 hi/lo)
    Z1 = M @ (P1h + P1l) + lam*b1 (x) sqrt(deg)             (PE, fp8 DR)
    h1' = relu(dinv^2 (.) Z1) = lam * dinv (.) h1     (ACT + Pool fused)
    P2' = h1' @ W2 (PE fp16) ; split (ACT/DVE) ; Z2 likewise (PE) ;
    out = relu(dinv/lam (.) Z2)                       (DVE fused) -> DMA

The rank-1 bias rides INSIDE the A-matmul: blocks are padded 1000 -> 8
chunks of 126 rows; padded row 125 of chunk 0 of the stationary P tile
holds lam*b (exact for layer 1; for layer 2 a host solve u^T W2 = lam*b2
routes it through the W-fold, error ~2% of the tiny bias). The matching
moving-side row of M^T holds sqrt(deg). The dst axis stays PLAIN
1000-wide (250-col matmul chunks into 500-col PSUM banks — no pad
columns swept); the epilogues bridge plain-in to aug-out (h1) with
w=125 / w=126 strided views.

Sharding: 8 cores = 4 chunks x 2 node-halves; each core owns 64 blocks
(32 block-pairs) of [1000, 64].

Per iteration p (engine queues in issue order):
  PE:   W-fold2[p-1] (8 fp16 mm) -> A1[p] (2x16 DR mm) -> A2[p-1]
  ACT:  hi-cast2[p-1] x2, relu-stage1[p] per bank
  DVE:  lo-sub2[p-1] x2, fused relu*dinv epilogue2[p-1] per bank
  Pool: u-col copy, fused relu*dinv epilogue1[p] per bank (SBUF only —
        Pool cannot read PSUM)
  SP:   p1 hi/lo loads, per-bank output stores (HWDGE; Pool-engine DMA
        triggers cost ~1us each so all DMA rides SP/scalar queues)
PSUM: pps (2 banks) + gps0/gps1 (2x2 banks).
"""
import sys
import numpy as np
import ml_dtypes

sys.path.insert(0, '/opt/trn_rl_repo')

import concourse.bass as bass  # noqa: E402
import concourse.bacc as bacc  # noqa: E402
import concourse.mybir as mybir  # noqa: E402
import concourse.tile as tile  # noqa: E402
from concourse.bass_utils import run_bass_kernel_spmd  # noqa: E402

N, T, D = 1000, 512, 64
CS = 128                 # timesteps per chunk
NCORES = 8
PAIRS = 32               # block-pairs per core (64 blocks of 1000 rows)
NA = 1008                # augmented row count: 8 chunks x 126
F8 = ml_dtypes.float8_e4m3
DR = mybir.MatmulPerfMode.DoubleRow
LAM = 32.0

_prog = None
LAST_RESULTS = None


def _build_program(skip=frozenset()):
    nc = bacc.Bacc(None)
    p1h = nc.declare_dram_parameter("p1h", [PAIRS, 128, 8 * 128],
                                    mybir.dt.float8e4, isOutput=False)
    p1l = nc.declare_dram_parameter("p1l", [PAIRS, 128, 8 * 128],
                                    mybir.dt.float8e4, isOutput=False)
    mth = nc.declare_dram_parameter("mth", [128, 8 * N], mybir.dt.float8e4,
                                    isOutput=False)
    wt2 = nc.declare_dram_parameter("wt2", [128, 128], mybir.dt.float16,
                                    isOutput=False)
    d2t = nc.declare_dram_parameter("d2t", [128, N], mybir.dt.float16,
                                    isOutput=False)
    dlt = nc.declare_dram_parameter("dlt", [128, N], mybir.dt.float16,
                                    isOutput=False)
    uct = nc.declare_dram_parameter("uct", [128, 8], mybir.dt.float16,
                                    isOutput=False)
    xout = nc.declare_dram_parameter("xout", [PAIRS, 128, N],
                                     mybir.dt.float16, isOutput=True)

    with tile.TileContext(nc) as tc:
        with tc.tile_pool(name="const", bufs=1) as cpool, \
             tc.tile_pool(name="work", bufs=2) as wpool, \
             tc.tile_pool(name="psA", bufs=1, space="PSUM") as psA, \
             tc.tile_pool(name="psB", bufs=1, space="PSUM") as psB:

            st = {}

            def load_p1(p, queues=None):
                """Layer-1 P tiles arrive pre-split from the host."""
                hi = wpool.tile([128, 8, 128], mybir.dt.float8e4,
                                name="hi0", tag="hi0", bufs=3)
                lo = wpool.tile([128, 8, 128], mybir.dt.float8e4,
                                name="lo0", tag="lo0", bufs=3)
                qh, ql = queues or (nc.sync, nc.sync)
                if "indma" not in skip:
                    qh.dma_start(hi.rearrange("p k w -> p (k w)")[:, :],
                                 p1h[p, :, :])
                    ql.dma_start(lo.rearrange("p k w -> p (k w)")[:, :],
                                 p1l[p, :, :])
                else:
                    qh.dma_start(hi[0:2, 0, 0:2], p1h[p, 0:2, 0:2])
                    ql.dma_start(lo[0:2, 0, 0:2], p1l[p, 0:2, 0:2])
                st[(p, 'sp0')] = (hi, lo)

            def wfold2(p):
                """8 fp16 matmuls: lhsT = 126-row h1' chunks, rhs = Wbig2."""
                src = st.pop((p, 'h1'))
                pps = [psA.tile([128, 512], mybir.dt.float32,
                                name=f"pps_{t}", tag=f"pps_{t}")
                       for t in range(2)]
                ww = 128 if "wfold" not in skip else 4
                for c in (4, 5, 6, 7, 0, 1, 2, 3):
                    nc.tensor.matmul(
                        pps[c // 4][0:126,
                                    128 * (c % 4):128 * (c % 4) + ww],
                        src[:, 126 * c:126 * c + 126], wt_t[:, 0:ww],
                        start=True, stop=True)
                st[(p, 'pps')] = pps

            def split2(p):
                """P2 psum -> fp8 hi sbuf tile [128, 8, 128]. Layer 2 runs
                a SINGLE fp8 product: the dropped Pl2 correction costs
                ~0.7e-2 rel err (measured 1.1e-2 total vs the 2e-2 gate)
                and saves 16 DR matmuls + 2 DVE subtracts per pair."""
                pps = st.pop((p, 'pps'))
                hi = wpool.tile([128, 8, 128], mybir.dt.float8e4,
                                name="hi1", tag="hi1", bufs=2)
                hv = hi.rearrange("p k w -> p (k w)")
                tw = 512 if "split" not in skip else 4
                for t in (1, 0):
                    nc.scalar.copy(hv[0:126, 512 * t:512 * t + tw],
                                   pps[t][0:126, 0:tw])
                st[(p, 'sp1')] = (hi, hi)

            def atype_epi(p, li):
                """Per bank: 16 DR matmuls (hi+lo products) then that bank's
                epilogue ops, so downstream consumers start half a pair
                early. L1 epi: ACT relu-stage + Pool fused max*mult into the
                aug h1 tile; L2 epi: DVE fused max*mult straight from PSUM,
                then that half's store."""
                hi, lo = st.pop((p, f'sp{li}'))
                gps = [psB.tile([128, 500], mybir.dt.float32,
                                name=f"gps{li}_{t}", tag=f"gps{li}_{t}")
                       for t in range(2)]
                aw = 250 if "atype" not in skip else 4
                rw = 125 if "epi" not in skip else 4
                last = (li == 1 and p == PAIRS - 1)
                if li == 0:
                    t = wpool.tile([128, NA], mybir.dt.float16, name='h1',
                                   tag='h1')
                    ov = t.rearrange("p (k w) -> p k w", w=126)
                    # u-row columns for the next W-fold (8 aug cols); on
                    # ACT so it is not queued behind Pool epilogue ops
                    nc.scalar.copy(ov[:, :, 125:126], uct_v[:, :, :])
                else:
                    t = wpool.tile([128, N], mybir.dt.float16, name='ot',
                                   tag='ot')
                    ov = t.rearrange("p (k w) -> p k w", w=125)
                if li == 0 and p == 0:
                    # pair 0: consume M quarters in DMA-arrival order
                    # (chunks 4-7 ride HWDGE and land ~0.5us before the
                    # SWDGE-carried 0-3), one m per group
                    groups = [(prod, m, b) for prod in (hi, lo)
                              for m in (2, 3, 0, 1) for b in (0, 1)]
                elif li == 0:
                    # L1 operands arrived by DMA long ago: bank-major so
                    # bank0 stops early for its epilogue chain
                    groups = [(hi, 0, 0), (hi, 2, 0), (lo, 0, 0), (lo, 2, 0),
                              (hi, 0, 1), (hi, 2, 1), (lo, 0, 1), (lo, 2, 1)]
                else:
                    # L2 single product: consume the t1 cast (ready first)
                    # before t0; bank1 last so the drain path overlaps
                    groups = [(hi, 2, 0), (hi, 2, 1),
                              (hi, 0, 0), (hi, 0, 1)]
                nmm = [0, 0]
                for prod, m0, bank in groups:
                    ms = (m0,) if (li == 0 and p == 0) else (m0, m0 + 1)
                    for ci in range(2):
                        for m in ms:
                            n0 = 500 * bank + 250 * ci
                            nc.tensor.matmul(
                                gps[bank][:, 250 * ci:250 * ci + aw],
                                prod[0:126, 2 * m:2 * m + 2, :],
                                mth_t[0:126, 2 * m:2 * m + 2, n0:n0 + aw],
                                start=(nmm[bank] == 0),
                                stop=(nmm[bank] == 15),
                                perf_mode=DR)
                            nmm[bank] += 1
                for bank in ((1, 0) if last else (0, 1)):
                    # fused relu+scale straight from PSUM on DVE for all
                    # four bank epilogues: with the L2 lo-subs gone, DVE
                    # has the headroom, and this retires the ACT-queued
                    # relu-stage + Pool-mult latency chain for h1-bank0
                    if True:
                        dv = d2T_v if li == 0 else dlT_v
                        nc.vector.scalar_tensor_tensor(
                            ov[:, 4 * bank:4 * bank + 4, 0:rw],
                            gps[bank].rearrange("p (k w) -> p k w",
                                                w=125)[:, 0:4, 0:rw],
                            0.0,
                            dv[:, 4 * bank:4 * bank + 4, 0:rw],
                            op0=mybir.AluOpType.max,
                            op1=mybir.AluOpType.mult)
                    if li == 1 and "outdma" not in skip:
                        # two queues so the final stores overlap; the last
                        # pair rides HWDGE + SWDGE so even the triggers
                        # run on different devices
                        q = nc.sync if bank == 0 else nc.scalar
                        q.dma_start(
                            xout[p, :, 500 * bank:500 * bank + 500],
                            t[:, 500 * bank:500 * bank + 500])
                name = 'h1' if li == 0 else 'ot'
                st[(p, name)] = t

            # ---- prologue: spread loads over the DMA queues so the
            # M-matrix and first P1 tiles (critical for A1[0]) arrive in
            # parallel ----
            mth_t = cpool.tile([128, 8, N], mybir.dt.float8e4, name="mth_t")
            mth_v = mth.rearrange("p (k n) -> p k n", k=8)
            wt_t = cpool.tile([128, 128], mybir.dt.float16, name="wt2")
            d2T = cpool.tile([128, N], mybir.dt.float16, name="d2T")
            dlT = cpool.tile([128, N], mybir.dt.float16, name="dlT")
            uct_t = cpool.tile([128, 8], mybir.dt.float16, name="uct_t")
            d2T_v = d2T.rearrange("p (k w) -> p k w", w=125)
            dlT_v = dlT.rearrange("p (k w) -> p k w", w=125)
            uct_v = uct_t.rearrange("p (k o) -> p k o", o=1)
            # p1[0] leads the HWDGE queues (smallest critical load); mth
            # rides SWDGE (Pool is idle at prologue) + the HWDGE queues
            load_p1(0, queues=(nc.sync, nc.scalar))
            nc.gpsimd.dma_start(mth_t[:, 0:2, :], mth_v[:, 0:2, :])
            nc.gpsimd.dma_start(mth_t[:, 2:4, :], mth_v[:, 2:4, :])
            nc.sync.dma_start(mth_t[:, 4:6, :], mth_v[:, 4:6, :])
            nc.scalar.dma_start(mth_t[:, 6:8, :], mth_v[:, 6:8, :])
            nc.sync.dma_start(uct_t[:, :], uct[:, :])
            nc.sync.dma_start(d2T[:, :], d2t[:, :])
            load_p1(1, queues=(nc.sync, nc.scalar))
            # ---- PE warm-up: zero matmuls bridge the DMA wait so pair 0
            # runs beyond the 3us p-state ramp at full clock ----
            wsrc = cpool.tile([128, 252], mybir.dt.float16, name="wsrc")
            nc.vector.memset(wsrc[:, :], 0.0)
            warm = psB.tile([128, 500], mybir.dt.float32,
                            name="gps0_0", tag="gps0_0")
            for _ in range(16 if "warm" not in skip else 1):
                nc.tensor.matmul(warm[:, 0:252], wsrc[:, 0:128],
                                 wsrc[:, 0:252], start=True, stop=True)
            # consts for iter 1+: delay so they don't steal early DMA slots
            with tc.tile_wait_until(0.006):
                nc.scalar.dma_start(wt_t[:, :], wt2[:, :])
                nc.scalar.dma_start(dlT[:, :], dlt[:, :])
            load_p1(2)

            # ---- steady state: the L2 W-fold chain leads each iteration
            # (its split feeds this iteration's last matmuls) ----
            for p in range(PAIRS + 1):
                if p >= 1:
                    wfold2(p - 1)         # needs h1[p-1] (prev iter)
                    split2(p - 1)
                if p < PAIRS:
                    atype_epi(p, 0)
                if p >= 1:
                    atype_epi(p - 1, 1)
                if p + 3 < PAIRS:
                    load_p1(p + 3)

    nc.compile()
    return nc


def _host_prep(x, edge_index, W1, b1, W2, b2):
    x = np.ascontiguousarray(np.asarray(x, dtype=np.float32))
    ei = np.asarray(edge_index)
    row, col = ei[0], ei[1]
    deg = np.zeros(N, np.float32)
    np.add.at(deg, col, 1.0)
    deg += 1.0
    dinv = (1.0 / np.sqrt(deg)).astype(np.float32)
    Mc = np.zeros((N, N), np.float32)
    np.add.at(Mc, (col, row), 1.0)
    Mc[np.arange(N), np.arange(N)] += 1.0
    MT = np.ascontiguousarray(Mc.T)                     # [src, dst]
    sd = (1.0 / dinv).astype(np.float32)                # sqrt(deg)

    # M^T packed into 8 chunks of 126 partitions, dst axis plain 1000.
    # Partition 125 of chunk 0 is the rank-1 bias rhs row (sqrt(deg)).
    pk = np.zeros((128, 8, N), np.float32)
    for k in range(8):
        pk[0:125, k, :] = MT[125 * k:125 * k + 125]
    pk[125, 0, :] = sd
    mth = np.ascontiguousarray(pk.astype(F8).reshape(128, 8 * N))

    W1_16 = np.asarray(W1).astype(np.float16)
    W2_16 = np.asarray(W2).astype(np.float16)
    wt2 = np.zeros((128, 128), np.float16)
    wt2[:64, :64] = W2_16
    wt2[64:, 64:] = W2_16
    u2 = np.linalg.solve(W2_16.astype(np.float64).T,
                         LAM * np.asarray(b2).astype(np.float64))
    uct = np.zeros((128, 8), np.float16)
    uct[:, 0] = np.tile(u2.astype(np.float16), 2)

    d2t = np.ascontiguousarray(np.broadcast_to(
        (dinv * dinv).astype(np.float16), (128, N)).copy())
    dlt = np.ascontiguousarray(np.broadcast_to(
        (dinv / LAM).astype(np.float16), (128, N)).copy())

    # Layer-1 P tiles: P1' = (lam*dinv (.) X)(fp16) @ W1(fp16), computed on
    # the host (an input-side linear fold), packed into the augmented
    # [q(126-part), k(8), (2b,f)] layout with the exact lam*b1 bias row,
    # and pre-split into fp8 hi/lo.
    scale = (LAM * dinv).astype(np.float32)
    bias_row = np.tile((LAM * np.asarray(b1)).astype(np.float32), 2)
    p1hs, p1ls = [], []
    for k in range(NCORES):
        c, hf = k // 2, k % 2
        slab = x[500 * hf:500 * hf + 500,
                 128 * c:128 * (c + 1), :].reshape(64000, D)
        xs = (slab.reshape(PAIRS, 2, N, D)
              * scale[None, None, :, None]).astype(np.float16)
        P = xs.astype(np.float32) @ W1_16.astype(np.float32)  # [P,2,N,64]
        # [pair, k, q, (2b,f)]
        Pa = np.zeros((PAIRS, 8, 126, 128), np.float32)
        Pa[:, :, 0:125, 0:64] = P[:, 0].reshape(PAIRS, 8, 125, 64)
        Pa[:, :, 0:125, 64:128] = P[:, 1].reshape(PAIRS, 8, 125, 64)
        Pa[:, 0, 125, :] = bias_row
        Pa = Pa.transpose(0, 2, 1, 3)                   # [pair, q, k, bf]
        hi = Pa.astype(F8)
        lo = (Pa - hi.astype(np.float32)).astype(F8)
        z = np.zeros((PAIRS, 2, 8, 128), F8)            # partitions 126/127
        p1hs.append(np.ascontiguousarray(np.concatenate(
            [hi, z], axis=1).reshape(PAIRS, 128, 8 * 128)))
        p1ls.append(np.ascontiguousarray(np.concatenate(
            [lo, z], axis=1).reshape(PAIRS, 128, 8 * 128)))
    return mth, wt2, uct, d2t, dlt, p1hs, p1ls


def kernel(x, edge_index, W1, b1, W2, b2):
    global _prog, LAST_RESULTS
    if _prog is None:
        _prog = _build_program()
    nc = _prog

    mth, wt2, uct, d2t, dlt, p1hs, p1ls = \
        _host_prep(x, edge_index, W1, b1, W2, b2)
    in_maps = [{"p1h": p1hs[k], "p1l": p1ls[k], "mth": mth, "wt2": wt2,
                "d2t": d2t, "dlt": dlt, "uct": uct}
               for k in range(NCORES)]

    LAST_RESULTS = run_bass_kernel_spmd(nc, in_maps,
                                        core_ids=list(range(NCORES)))

    out = np.empty((N, T, D), np.float32)
    for k in range(NCORES):
        c, hf = k // 2, k % 2
        ot = LAST_RESULTS.results[k]["xout"]          # [PAIRS, 128, N] f16
        # [pair, (2b, f), row] -> rows [pair*2+b]*1000+row, feat f
        slab = ot.reshape(PAIRS, 2, D, N).transpose(0, 1, 3, 2) \
                 .reshape(64000, D).astype(np.float32)
        out[500 * hf:500 * hf + 500, 128 * c:128 * (c + 1), :] = \
            slab.reshape(500, CS, D)
    return out


# revision 54
# speedup vs baseline: 1.0080x; 1.0080x over previous
"""Trainium2 Bass kernel for ChunkedTGnnModel — exact-integer-M fp8 design.

Math (per temporal chunk, flattened to a [128000, 64] slab whose
consecutive 1000-row blocks each see the same dense normalized adjacency
A_hat = D^-1/2 (Adj + I) D^-1/2, [1000 x 1000]):

    out = relu(A_hat @ (block @ W1) + b1)   (layer 2 same)

Core trick: fold the D^-1/2 diagonals OUT of the matmul. The streamed
matrix becomes M = Adj + I whose entries are small INTEGERS ({0,1,2}) —
exactly representable in fp8e4m3. Only the P = X@W operand needs an
error split (P = Ph + Pl), so the dominant A-type matmul needs 2 fp8
DoubleRow products instead of 3 (as an fp8 split of the full A would).

Folding details (lambda = 32 keeps fp8 operands out of the subnormal
range; all scale hops are powers of two except the dinv vectors):
    P1' = (lam * dinv (.) X) @ W1   (host, fused into input packing,
                                     shipped pre-split as fp8 hi/lo)
    Z1 = M @ (P1h + P1l) + lam*b1 (x) sqrt(deg)             (PE, fp8 DR)
    h1' = relu(dinv^2 (.) Z1) = lam * dinv (.) h1     (ACT + Pool fused)
    P2' = h1' @ W2 (PE fp16) ; split (ACT/DVE) ; Z2 likewise (PE) ;
    out = relu(dinv/lam (.) Z2)                       (DVE fused) -> DMA

The rank-1 bias rides INSIDE the A-matmul: blocks are padded 1000 -> 8
chunks of 126 rows; padded row 125 of chunk 0 of the stationary P tile
holds lam*b (exact for layer 1; for layer 2 a host solve u^T W2 = lam*b2
routes it through the W-fold, error ~2% of the tiny bias). The matching
moving-side row of M^T holds sqrt(deg). The dst axis stays PLAIN
1000-wide (250-col matmul chunks into 500-col PSUM banks — no pad
columns swept); the epilogues bridge plain-in to aug-out (h1) with
w=125 / w=126 strided views.

Sharding: 8 cores = 4 chunks x 2 node-halves; each core owns 64 blocks
(32 block-pairs) of [1000, 64].

Per iteration p (engine queues in issue order):
  PE:   W-fold2[p-1] (8 fp16 mm) -> A1[p] (2x16 DR mm) -> A2[p-1]
  ACT:  hi-cast2[p-1] x2, relu-stage1[p] per bank
  DVE:  lo-sub2[p-1] x2, fused relu*dinv epilogue2[p-1] per bank
  Pool: u-col copy, fused relu*dinv epilogue1[p] per bank (SBUF only —
        Pool cannot read PSUM)
  SP:   p1 hi/lo loads, per-bank output stores (HWDGE; Pool-engine DMA
        triggers cost ~1us each so all DMA rides SP/scalar queues)
PSUM: pps (2 banks) + gps0/gps1 (2x2 banks).
"""
import sys
import numpy as np
import ml_dtypes

sys.path.insert(0, '/opt/trn_rl_repo')

import concourse.bass as bass  # noqa: E402
import concourse.bacc as bacc  # noqa: E402
import concourse.mybir as mybir  # noqa: E402
import concourse.tile as tile  # noqa: E402
from concourse.bass_utils import run_bass_kernel_spmd  # noqa: E402

N, T, D = 1000, 512, 64
CS = 128                 # timesteps per chunk
NCORES = 8
PAIRS = 32               # block-pairs per core (64 blocks of 1000 rows)
NA = 1008                # augmented row count: 8 chunks x 126
F8 = ml_dtypes.float8_e4m3
DR = mybir.MatmulPerfMode.DoubleRow
LAM = 32.0

_prog = None
LAST_RESULTS = None


def _build_program(skip=frozenset()):
    nc = bacc.Bacc(None)
    p1h = nc.declare_dram_parameter("p1h", [PAIRS, 128, 8 * 128],
                                    mybir.dt.float8e4, isOutput=False)
    p1l = nc.declare_dram_parameter("p1l", [PAIRS, 128, 8 * 128],
                                    mybir.dt.float8e4, isOutput=False)
    mth = nc.declare_dram_parameter("mth", [128, 8 * N], mybir.dt.float8e4,
                                    isOutput=False)
    wt2 = nc.declare_dram_parameter("wt2", [128, 128], mybir.dt.float16,
                                    isOutput=False)
    d2t = nc.declare_dram_parameter("d2t", [128, N], mybir.dt.float16,
                                    isOutput=False)
    dlt = nc.declare_dram_parameter("dlt", [128, N], mybir.dt.float16,
                                    isOutput=False)
    uct = nc.declare_dram_parameter("uct", [128, 8], mybir.dt.float16,
                                    isOutput=False)
    xout = nc.declare_dram_parameter("xout", [PAIRS, 128, N],
                                     mybir.dt.float16, isOutput=True)

    with tile.TileContext(nc) as tc:
        with tc.tile_pool(name="const", bufs=1) as cpool, \
             tc.tile_pool(name="work", bufs=2) as wpool, \
             tc.tile_pool(name="psA", bufs=1, space="PSUM") as psA, \
             tc.tile_pool(name="psB", bufs=1, space="PSUM") as psB:

            st = {}

            def load_p1(p, queues=None):
                """Layer-1 P tiles arrive pre-split from the host."""
                hi = wpool.tile([128, 8, 128], mybir.dt.float8e4,
                                name="hi0", tag="hi0", bufs=3)
                lo = wpool.tile([128, 8, 128], mybir.dt.float8e4,
                                name="lo0", tag="lo0", bufs=3)
                qh, ql = queues or (nc.sync, nc.sync)
                if "indma" not in skip:
                    qh.dma_start(hi.rearrange("p k w -> p (k w)")[:, :],
                                 p1h[p, :, :])
                    ql.dma_start(lo.rearrange("p k w -> p (k w)")[:, :],
                                 p1l[p, :, :])
                else:
                    qh.dma_start(hi[0:2, 0, 0:2], p1h[p, 0:2, 0:2])
                    ql.dma_start(lo[0:2, 0, 0:2], p1l[p, 0:2, 0:2])
                st[(p, 'sp0')] = (hi, lo)

            def wfold2(p):
                """8 fp16 matmuls: lhsT = 126-row h1' chunks, rhs = Wbig2."""
                src = st.pop((p, 'h1'))
                pps = [psA.tile([128, 512], mybir.dt.float32,
                                name=f"pps_{t}", tag=f"pps_{t}")
                       for t in range(2)]
                ww = 128 if "wfold" not in skip else 4
                for c in (4, 5, 6, 7, 0, 1, 2, 3):
                    nc.tensor.matmul(
                        pps[c // 4][0:126,
                                    128 * (c % 4):128 * (c % 4) + ww],
                        src[:, 126 * c:126 * c + 126], wt_t[:, 0:ww],
                        start=True, stop=True)
                st[(p, 'pps')] = pps

            def split2(p):
                """P2 psum -> fp8 hi sbuf tile [128, 8, 128]. Layer 2 runs
                a SINGLE fp8 product: the dropped Pl2 correction costs
                ~0.7e-2 rel err (measured 1.1e-2 total vs the 2e-2 gate)
                and saves 16 DR matmuls + 2 DVE subtracts per pair."""
                pps = st.pop((p, 'pps'))
                hi = wpool.tile([128, 8, 128], mybir.dt.float8e4,
                                name="hi1", tag="hi1", bufs=2)
                hv = hi.rearrange("p k w -> p (k w)")
                tw = 512 if "split" not in skip else 4
                for t in (1, 0):
                    nc.scalar.copy(hv[0:126, 512 * t:512 * t + tw],
                                   pps[t][0:126, 0:tw])
                st[(p, 'sp1')] = (hi, hi)

            def atype_epi(p, li):
                """Per bank: 16 DR matmuls (hi+lo products) then that bank's
                epilogue ops, so downstream consumers start half a pair
                early. L1 epi: ACT relu-stage + Pool fused max*mult into the
                aug h1 tile; L2 epi: DVE fused max*mult straight from PSUM,
                then that half's store."""
                hi, lo = st.pop((p, f'sp{li}'))
                # L1 PSUM alternates by pair parity: the next pair's
                # accumulation no longer waits on THIS pair's epilogue
                # read (a ~100ns/iter loop-latency hazard); 4+2+2 banks.
                tg = f"gps0{p % 2}" if li == 0 else "gps1"
                gps = [psB.tile([128, 500], mybir.dt.float32,
                                name=f"{tg}_{t}", tag=f"{tg}_{t}")
                       for t in range(2)]
                aw = 250 if "atype" not in skip else 4
                rw = 125 if "epi" not in skip else 4
                last = (li == 1 and p == PAIRS - 1)
                if li == 0:
                    t = wpool.tile([128, NA], mybir.dt.float16, name='h1',
                                   tag='h1')
                    ov = t.rearrange("p (k w) -> p k w", w=126)
                    # u-row columns for the next W-fold (8 aug cols); on
                    # ACT so it is not queued behind Pool epilogue ops
                    nc.scalar.copy(ov[:, :, 125:126], uct_v[:, :, :])
                else:
                    t = wpool.tile([128, N], mybir.dt.float16, name='ot',
                                   tag='ot')
                    ov = t.rearrange("p (k w) -> p k w", w=125)
                if li == 0 and p == 0:
                    # pair 0: consume M quarters in DMA-arrival order
                    # (chunks 4-7 ride HWDGE and land ~0.5us before the
                    # SWDGE-carried 0-3), one m per group
                    groups = [(prod, m, b) for prod in (hi, lo)
                              for m in (2, 3, 0, 1) for b in (0, 1)]
                elif li == 0:
                    # L1 operands arrived by DMA long ago: bank-major so
                    # bank0 stops early for its epilogue chain
                    groups = [(hi, 0, 0), (hi, 2, 0), (lo, 0, 0), (lo, 2, 0),
                              (hi, 0, 1), (hi, 2, 1), (lo, 0, 1), (lo, 2, 1)]
                else:
                    # L2 single product: consume the t1 cast (ready first)
                    # before t0; bank1 last so the drain path overlaps
                    groups = [(hi, 2, 0), (hi, 2, 1),
                              (hi, 0, 0), (hi, 0, 1)]
                nmm = [0, 0]
                for prod, m0, bank in groups:
                    ms = (m0,) if (li == 0 and p == 0) else (m0, m0 + 1)
                    for ci in range(2):
                        for m in ms:
                            n0 = 500 * bank + 250 * ci
                            nc.tensor.matmul(
                                gps[bank][:, 250 * ci:250 * ci + aw],
                                prod[0:126, 2 * m:2 * m + 2, :],
                                mth_t[0:126, 2 * m:2 * m + 2, n0:n0 + aw],
                                start=(nmm[bank] == 0),
                                stop=(nmm[bank] == 15),
                                perf_mode=DR)
                            nmm[bank] += 1
                for bank in ((1, 0) if last else (0, 1)):
                    # fused relu+scale straight from PSUM on DVE for all
                    # four bank epilogues: with the L2 lo-subs gone, DVE
                    # has the headroom, and this retires the ACT-queued
                    # relu-stage + Pool-mult latency chain for h1-bank0
                    if True:
                        dv = d2T_v if li == 0 else dlT_v
                        nc.vector.scalar_tensor_tensor(
                            ov[:, 4 * bank:4 * bank + 4, 0:rw],
                            gps[bank].rearrange("p (k w) -> p k w",
                                                w=125)[:, 0:4, 0:rw],
                            0.0,
                            dv[:, 4 * bank:4 * bank + 4, 0:rw],
                            op0=mybir.AluOpType.max,
                            op1=mybir.AluOpType.mult)
                    if li == 1 and "outdma" not in skip:
                        # two queues so the final stores overlap; the last
                        # pair rides HWDGE + SWDGE so even the triggers
                        # run on different devices
                        q = nc.sync if bank == 0 else nc.scalar
                        q.dma_start(
                            xout[p, :, 500 * bank:500 * bank + 500],
                            t[:, 500 * bank:500 * bank + 500])
                name = 'h1' if li == 0 else 'ot'
                st[(p, name)] = t

            # ---- prologue: spread loads over the DMA queues so the
            # M-matrix and first P1 tiles (critical for A1[0]) arrive in
            # parallel ----
            mth_t = cpool.tile([128, 8, N], mybir.dt.float8e4, name="mth_t")
            mth_v = mth.rearrange("p (k n) -> p k n", k=8)
            wt_t = cpool.tile([128, 128], mybir.dt.float16, name="wt2")
            d2T = cpool.tile([128, N], mybir.dt.float16, name="d2T")
            dlT = cpool.tile([128, N], mybir.dt.float16, name="dlT")
            uct_t = cpool.tile([128, 8], mybir.dt.float16, name="uct_t")
            d2T_v = d2T.rearrange("p (k w) -> p k w", w=125)
            dlT_v = dlT.rearrange("p (k w) -> p k w", w=125)
            uct_v = uct_t.rearrange("p (k o) -> p k o", o=1)
            # p1[0] leads the HWDGE queues (smallest critical load); mth
            # rides SWDGE (Pool is idle at prologue) + the HWDGE queues
            load_p1(0, queues=(nc.sync, nc.scalar))
            nc.gpsimd.dma_start(mth_t[:, 0:2, :], mth_v[:, 0:2, :])
            nc.gpsimd.dma_start(mth_t[:, 2:4, :], mth_v[:, 2:4, :])
            nc.sync.dma_start(mth_t[:, 4:6, :], mth_v[:, 4:6, :])
            nc.scalar.dma_start(mth_t[:, 6:8, :], mth_v[:, 6:8, :])
            nc.sync.dma_start(uct_t[:, :], uct[:, :])
            nc.sync.dma_start(d2T[:, :], d2t[:, :])
            load_p1(1, queues=(nc.sync, nc.scalar))
            # ---- PE warm-up: zero matmuls bridge the DMA wait so pair 0
            # runs beyond the 3us p-state ramp at full clock ----
            wsrc = cpool.tile([128, 252], mybir.dt.float16, name="wsrc")
            nc.vector.memset(wsrc[:, :], 0.0)
            warm = psB.tile([128, 500], mybir.dt.float32,
                            name="gps00_0", tag="gps00_0")
            for _ in range(16 if "warm" not in skip else 1):
                nc.tensor.matmul(warm[:, 0:252], wsrc[:, 0:128],
                                 wsrc[:, 0:252], start=True, stop=True)
            # consts for iter 1+: delay so they don't steal early DMA slots
            with tc.tile_wait_until(0.006):
                nc.scalar.dma_start(wt_t[:, :], wt2[:, :])
                nc.scalar.dma_start(dlT[:, :], dlt[:, :])
            load_p1(2)

            # ---- steady state: the L2 W-fold chain leads each iteration
            # (its split feeds this iteration's last matmuls) ----
            for p in range(PAIRS + 1):
                if p >= 1:
                    wfold2(p - 1)         # needs h1[p-1] (prev iter)
                    split2(p - 1)
                if p < PAIRS:
                    atype_epi(p, 0)
                if p >= 1:
                    atype_epi(p - 1, 1)
                if p + 3 < PAIRS:
                    load_p1(p + 3)

    nc.compile()
    return nc


def _host_prep(x, edge_index, W1, b1, W2, b2):
    x = np.ascontiguousarray(np.asarray(x, dtype=np.float32))
    ei = np.asarray(edge_index)
    row, col = ei[0], ei[1]
    deg = np.zeros(N, np.float32)
    np.add.at(deg, col, 1.0)
    deg += 1.0
    dinv = (1.0 / np.sqrt(deg)).astype(np.float32)
    Mc = np.zeros((N, N), np.float32)
    np.add.at(Mc, (col, row), 1.0)
    Mc[np.arange(N), np.arange(N)] += 1.0
    MT = np.ascontiguousarray(Mc.T)                     # [src, dst]
    sd = (1.0 / dinv).astype(np.float32)                # sqrt(deg)

    # M^T packed into 8 chunks of 126 partitions, dst axis plain 1000.
    # Partition 125 of chunk 0 is the rank-1 bias rhs row (sqrt(deg)).
    pk = np.zeros((128, 8, N), np.float32)
    for k in range(8):
        pk[0:125, k, :] = MT[125 * k:125 * k + 125]
    pk[125, 0, :] = sd
    mth = np.ascontiguousarray(pk.astype(F8).reshape(128, 8 * N))

    W1_16 = np.asarray(W1).astype(np.float16)
    W2_16 = np.asarray(W2).astype(np.float16)
    wt2 = np.zeros((128, 128), np.float16)
    wt2[:64, :64] = W2_16
    wt2[64:, 64:] = W2_16
    u2 = np.linalg.solve(W2_16.astype(np.float64).T,
                         LAM * np.asarray(b2).astype(np.float64))
    uct = np.zeros((128, 8), np.float16)
    uct[:, 0] = np.tile(u2.astype(np.float16), 2)

    d2t = np.ascontiguousarray(np.broadcast_to(
        (dinv * dinv).astype(np.float16), (128, N)).copy())
    dlt = np.ascontiguousarray(np.broadcast_to(
        (dinv / LAM).astype(np.float16), (128, N)).copy())

    # Layer-1 P tiles: P1' = (lam*dinv (.) X)(fp16) @ W1(fp16), computed on
    # the host (an input-side linear fold), packed into the augmented
    # [q(126-part), k(8), (2b,f)] layout with the exact lam*b1 bias row,
    # and pre-split into fp8 hi/lo.
    scale = (LAM * dinv).astype(np.float32)
    bias_row = np.tile((LAM * np.asarray(b1)).astype(np.float32), 2)
    p1hs, p1ls = [], []
    for k in range(NCORES):
        c, hf = k // 2, k % 2
        slab = x[500 * hf:500 * hf + 500,
                 128 * c:128 * (c + 1), :].reshape(64000, D)
        xs = (slab.reshape(PAIRS, 2, N, D)
              * scale[None, None, :, None]).astype(np.float16)
        P = xs.astype(np.float32) @ W1_16.astype(np.float32)  # [P,2,N,64]
        # [pair, k, q, (2b,f)]
        Pa = np.zeros((PAIRS, 8, 126, 128), np.float32)
        Pa[:, :, 0:125, 0:64] = P[:, 0].reshape(PAIRS, 8, 125, 64)
        Pa[:, :, 0:125, 64:128] = P[:, 1].reshape(PAIRS, 8, 125, 64)
        Pa[:, 0, 125, :] = bias_row
        Pa = Pa.transpose(0, 2, 1, 3)                   # [pair, q, k, bf]
        hi = Pa.astype(F8)
        lo = (Pa - hi.astype(np.float32)).astype(F8)
        z = np.zeros((PAIRS, 2, 8, 128), F8)            # partitions 126/127
        p1hs.append(np.ascontiguousarray(np.concatenate(
            [hi, z], axis=1).reshape(PAIRS, 128, 8 * 128)))
        p1ls.append(np.ascontiguousarray(np.concatenate(
            [lo, z], axis=1).reshape(PAIRS, 128, 8 * 128)))
    return mth, wt2, uct, d2t, dlt, p1hs, p1ls


def kernel(x, edge_index, W1, b1, W2, b2):
    global _prog, LAST_RESULTS
    if _prog is None:
        _prog = _build_program()
    nc = _prog

    mth, wt2, uct, d2t, dlt, p1hs, p1ls = \
        _host_prep(x, edge_index, W1, b1, W2, b2)
    in_maps = [{"p1h": p1hs[k], "p1l": p1ls[k], "mth": mth, "wt2": wt2,
                "d2t": d2t, "dlt": dlt, "uct": uct}
               for k in range(NCORES)]

    LAST_RESULTS = run_bass_kernel_spmd(nc, in_maps,
                                        core_ids=list(range(NCORES)))

    out = np.empty((N, T, D), np.float32)
    for k in range(NCORES):
        c, hf = k // 2, k % 2
        ot = LAST_RESULTS.results[k]["xout"]          # [PAIRS, 128, N] f16
        # [pair, (2b, f), row] -> rows [pair*2+b]*1000+row, feat f
        slab = ot.reshape(PAIRS, 2, D, N).transpose(0, 1, 3, 2) \
                 .reshape(64000, D).astype(np.float32)
        out[500 * hf:500 * hf + 500, 128 * c:128 * (c + 1), :] = \
            slab.reshape(500, CS, D)
    return out


# revision 55
# speedup vs baseline: 1.0302x; 1.0220x over previous
"""Trainium2 Bass kernel for ChunkedTGnnModel — exact-integer-M fp8 design.

Math (per temporal chunk, flattened to a [128000, 64] slab whose
consecutive 1000-row blocks each see the same dense normalized adjacency
A_hat = D^-1/2 (Adj + I) D^-1/2, [1000 x 1000]):

    out = relu(A_hat @ (block @ W1) + b1)   (layer 2 same)

Core trick: fold the D^-1/2 diagonals OUT of the matmul. The streamed
matrix becomes M = Adj + I whose entries are small INTEGERS ({0,1,2}) —
exactly representable in fp8e4m3. Only the P = X@W operand needs an
error split (P = Ph + Pl), so the dominant A-type matmul needs 2 fp8
DoubleRow products instead of 3 (as an fp8 split of the full A would).

Folding details (lambda = 32 keeps fp8 operands out of the subnormal
range; all scale hops are powers of two except the dinv vectors):
    P1' = (lam * dinv (.) X) @ W1   (host, fused into input packing,
                                     shipped pre-split as fp8 hi/lo)
    Z1 = M @ (P1h + P1l) + lam*b1 (x) sqrt(deg)             (PE, fp8 DR)
    h1' = relu(dinv^2 (.) Z1) = lam * dinv (.) h1     (ACT + Pool fused)
    P2' = h1' @ W2 (PE fp16) ; split (ACT/DVE) ; Z2 likewise (PE) ;
    out = relu(dinv/lam (.) Z2)                       (DVE fused) -> DMA

The rank-1 bias rides INSIDE the A-matmul: blocks are padded 1000 -> 8
chunks of 126 rows; padded row 125 of chunk 0 of the stationary P tile
holds lam*b (exact for layer 1; for layer 2 a host solve u^T W2 = lam*b2
routes it through the W-fold, error ~2% of the tiny bias). The matching
moving-side row of M^T holds sqrt(deg). The dst axis stays PLAIN
1000-wide (250-col matmul chunks into 500-col PSUM banks — no pad
columns swept); the epilogues bridge plain-in to aug-out (h1) with
w=125 / w=126 strided views.

Sharding: 8 cores = 4 chunks x 2 node-halves; each core owns 64 blocks
(32 block-pairs) of [1000, 64].

Per iteration p (engine queues in issue order):
  PE:   W-fold2[p-1] (8 fp16 mm) -> A1[p] (2x16 DR mm) -> A2[p-1]
  ACT:  hi-cast2[p-1] x2, relu-stage1[p] per bank
  DVE:  lo-sub2[p-1] x2, fused relu*dinv epilogue2[p-1] per bank
  Pool: u-col copy, fused relu*dinv epilogue1[p] per bank (SBUF only —
        Pool cannot read PSUM)
  SP:   p1 hi/lo loads, per-bank output stores (HWDGE; Pool-engine DMA
        triggers cost ~1us each so all DMA rides SP/scalar queues)
PSUM: pps (2 banks) + gps0/gps1 (2x2 banks).
"""
import sys
import numpy as np
import ml_dtypes

sys.path.insert(0, '/opt/trn_rl_repo')

import concourse.bass as bass  # noqa: E402
import concourse.bacc as bacc  # noqa: E402
import concourse.mybir as mybir  # noqa: E402
import concourse.tile as tile  # noqa: E402
from concourse.bass_utils import run_bass_kernel_spmd  # noqa: E402

N, T, D = 1000, 512, 64
CS = 128                 # timesteps per chunk
NCORES = 8
PAIRS = 32               # block-pairs per core (64 blocks of 1000 rows)
NA = 1008                # augmented row count: 8 chunks x 126
F8 = ml_dtypes.float8_e4m3
DR = mybir.MatmulPerfMode.DoubleRow
LAM = 32.0

_prog = None
LAST_RESULTS = None


def _build_program(skip=frozenset()):
    nc = bacc.Bacc(None)
    p1h = nc.declare_dram_parameter("p1h", [PAIRS, 128, 8 * 128],
                                    mybir.dt.float8e4, isOutput=False)
    p1l = nc.declare_dram_parameter("p1l", [PAIRS, 128, 8 * 128],
                                    mybir.dt.float8e4, isOutput=False)
    mth = nc.declare_dram_parameter("mth", [128, 8 * N], mybir.dt.float8e4,
                                    isOutput=False)
    wt2 = nc.declare_dram_parameter("wt2", [128, 128], mybir.dt.float16,
                                    isOutput=False)
    d2t = nc.declare_dram_parameter("d2t", [128, N], mybir.dt.float16,
                                    isOutput=False)
    dlt = nc.declare_dram_parameter("dlt", [128, N], mybir.dt.float16,
                                    isOutput=False)
    uct = nc.declare_dram_parameter("uct", [128, 8], mybir.dt.float16,
                                    isOutput=False)
    xout = nc.declare_dram_parameter("xout", [PAIRS, 128, N],
                                     mybir.dt.float16, isOutput=True)

    with tile.TileContext(nc) as tc:
        with tc.tile_pool(name="const", bufs=1) as cpool, \
             tc.tile_pool(name="work", bufs=2) as wpool, \
             tc.tile_pool(name="psA", bufs=1, space="PSUM") as psA, \
             tc.tile_pool(name="psB", bufs=1, space="PSUM") as psB:

            st = {}

            def load_p1(p, queues=None):
                """Layer-1 P tiles arrive pre-split from the host."""
                hi = wpool.tile([128, 8, 128], mybir.dt.float8e4,
                                name="hi0", tag="hi0", bufs=3)
                lo = wpool.tile([128, 8, 128], mybir.dt.float8e4,
                                name="lo0", tag="lo0", bufs=3)
                qh, ql = queues or (nc.sync, nc.sync)
                if "indma" not in skip:
                    qh.dma_start(hi.rearrange("p k w -> p (k w)")[:, :],
                                 p1h[p, :, :])
                    ql.dma_start(lo.rearrange("p k w -> p (k w)")[:, :],
                                 p1l[p, :, :])
                else:
                    qh.dma_start(hi[0:2, 0, 0:2], p1h[p, 0:2, 0:2])
                    ql.dma_start(lo[0:2, 0, 0:2], p1l[p, 0:2, 0:2])
                st[(p, 'sp0')] = (hi, lo)

            def wfold2(p):
                """8 fp16 matmuls: lhsT = 126-row h1' chunks, rhs = Wbig2."""
                src = st.pop((p, 'h1'))
                pps = [psA.tile([128, 512], mybir.dt.float32,
                                name=f"pps_{t}", tag=f"pps_{t}")
                       for t in range(2)]
                ww = 128 if "wfold" not in skip else 4
                # consume b0-backed h1 chunks first: with all-DVE
                # epilogues, bank1's stt is the one that lands last
                for c in range(8):
                    nc.tensor.matmul(
                        pps[c // 4][0:126,
                                    128 * (c % 4):128 * (c % 4) + ww],
                        src[:, 126 * c:126 * c + 126], wt_t[:, 0:ww],
                        start=True, stop=True)
                st[(p, 'pps')] = pps

            def split2(p):
                """P2 psum -> fp8 hi sbuf tile [128, 8, 128]. Layer 2 runs
                a SINGLE fp8 product: the dropped Pl2 correction costs
                ~0.7e-2 rel err (measured 1.1e-2 total vs the 2e-2 gate)
                and saves 16 DR matmuls + 2 DVE subtracts per pair."""
                pps = st.pop((p, 'pps'))
                hi = wpool.tile([128, 8, 128], mybir.dt.float8e4,
                                name="hi1", tag="hi1", bufs=2)
                hv = hi.rearrange("p k w -> p (k w)")
                tw = 512 if "split" not in skip else 4
                for t in (1, 0):
                    nc.scalar.copy(hv[0:126, 512 * t:512 * t + tw],
                                   pps[t][0:126, 0:tw])
                st[(p, 'sp1')] = (hi, hi)

            def atype_epi(p, li):
                """Per bank: 16 DR matmuls (hi+lo products) then that bank's
                epilogue ops, so downstream consumers start half a pair
                early. L1 epi: ACT relu-stage + Pool fused max*mult into the
                aug h1 tile; L2 epi: DVE fused max*mult straight from PSUM,
                then that half's store."""
                hi, lo = st.pop((p, f'sp{li}'))
                # L1 PSUM alternates by pair parity: the next pair's
                # accumulation no longer waits on THIS pair's epilogue
                # read (a ~100ns/iter loop-latency hazard); 4+2+2 banks.
                tg = f"gps0{p % 2}" if li == 0 else "gps1"
                gps = [psB.tile([128, 500], mybir.dt.float32,
                                name=f"{tg}_{t}", tag=f"{tg}_{t}")
                       for t in range(2)]
                aw = 250 if "atype" not in skip else 4
                rw = 125 if "epi" not in skip else 4
                last = (li == 1 and p == PAIRS - 1)
                if li == 0:
                    t = wpool.tile([128, NA], mybir.dt.float16, name='h1',
                                   tag='h1')
                    ov = t.rearrange("p (k w) -> p k w", w=126)
                    # u-row columns for the next W-fold (8 aug cols); on
                    # ACT so it is not queued behind Pool epilogue ops
                    nc.scalar.copy(ov[:, :, 125:126], uct_v[:, :, :])
                else:
                    t = wpool.tile([128, N], mybir.dt.float16, name='ot',
                                   tag='ot')
                    ov = t.rearrange("p (k w) -> p k w", w=125)
                if li == 0 and p == 0:
                    # pair 0: consume M quarters in DMA-arrival order
                    # (chunks 4-7 ride HWDGE and land ~0.5us before the
                    # SWDGE-carried 0-3), one m per group
                    groups = [(prod, m, b) for prod in (hi, lo)
                              for m in (2, 3, 0, 1) for b in (0, 1)]
                elif li == 0:
                    # L1 operands arrived by DMA long ago: bank-major so
                    # bank0 stops early for its epilogue chain
                    groups = [(hi, 0, 0), (hi, 2, 0), (lo, 0, 0), (lo, 2, 0),
                              (hi, 0, 1), (hi, 2, 1), (lo, 0, 1), (lo, 2, 1)]
                else:
                    # L2 single product: consume the t1 cast (ready first)
                    # before t0; bank1 last so the drain path overlaps
                    groups = [(hi, 2, 0), (hi, 2, 1),
                              (hi, 0, 0), (hi, 0, 1)]
                nmm = [0, 0]
                for prod, m0, bank in groups:
                    ms = (m0,) if (li == 0 and p == 0) else (m0, m0 + 1)
                    for ci in range(2):
                        for m in ms:
                            n0 = 500 * bank + 250 * ci
                            nc.tensor.matmul(
                                gps[bank][:, 250 * ci:250 * ci + aw],
                                prod[0:126, 2 * m:2 * m + 2, :],
                                mth_t[0:126, 2 * m:2 * m + 2, n0:n0 + aw],
                                start=(nmm[bank] == 0),
                                stop=(nmm[bank] == 15),
                                perf_mode=DR)
                            nmm[bank] += 1
                for bank in ((1, 0) if last else (0, 1)):
                    # fused relu+scale straight from PSUM on DVE for all
                    # four bank epilogues: with the L2 lo-subs gone, DVE
                    # has the headroom, and this retires the ACT-queued
                    # relu-stage + Pool-mult latency chain for h1-bank0
                    if True:
                        dv = d2T_v if li == 0 else dlT_v
                        nc.vector.scalar_tensor_tensor(
                            ov[:, 4 * bank:4 * bank + 4, 0:rw],
                            gps[bank].rearrange("p (k w) -> p k w",
                                                w=125)[:, 0:4, 0:rw],
                            0.0,
                            dv[:, 4 * bank:4 * bank + 4, 0:rw],
                            op0=mybir.AluOpType.max,
                            op1=mybir.AluOpType.mult)
                    if li == 1 and "outdma" not in skip:
                        # two queues so the final stores overlap; the last
                        # pair rides HWDGE + SWDGE so even the triggers
                        # run on different devices
                        q = nc.sync if bank == 0 else nc.scalar
                        q.dma_start(
                            xout[p, :, 500 * bank:500 * bank + 500],
                            t[:, 500 * bank:500 * bank + 500])
                name = 'h1' if li == 0 else 'ot'
                st[(p, name)] = t

            # ---- prologue: spread loads over the DMA queues so the
            # M-matrix and first P1 tiles (critical for A1[0]) arrive in
            # parallel ----
            mth_t = cpool.tile([128, 8, N], mybir.dt.float8e4, name="mth_t")
            mth_v = mth.rearrange("p (k n) -> p k n", k=8)
            wt_t = cpool.tile([128, 128], mybir.dt.float16, name="wt2")
            d2T = cpool.tile([128, N], mybir.dt.float16, name="d2T")
            dlT = cpool.tile([128, N], mybir.dt.float16, name="dlT")
            uct_t = cpool.tile([128, 8], mybir.dt.float16, name="uct_t")
            d2T_v = d2T.rearrange("p (k w) -> p k w", w=125)
            dlT_v = dlT.rearrange("p (k w) -> p k w", w=125)
            uct_v = uct_t.rearrange("p (k o) -> p k o", o=1)
            # p1[0] leads the HWDGE queues (smallest critical load); mth
            # rides SWDGE (Pool is idle at prologue) + the HWDGE queues
            load_p1(0, queues=(nc.sync, nc.scalar))
            nc.gpsimd.dma_start(mth_t[:, 0:2, :], mth_v[:, 0:2, :])
            nc.gpsimd.dma_start(mth_t[:, 2:4, :], mth_v[:, 2:4, :])
            nc.sync.dma_start(mth_t[:, 4:6, :], mth_v[:, 4:6, :])
            nc.scalar.dma_start(mth_t[:, 6:8, :], mth_v[:, 6:8, :])
            nc.sync.dma_start(uct_t[:, :], uct[:, :])
            nc.sync.dma_start(d2T[:, :], d2t[:, :])
            load_p1(1, queues=(nc.sync, nc.scalar))
            # ---- PE warm-up: zero matmuls bridge the DMA wait so pair 0
            # runs beyond the 3us p-state ramp at full clock ----
            wsrc = cpool.tile([128, 252], mybir.dt.float16, name="wsrc")
            nc.vector.memset(wsrc[:, :], 0.0)
            warm = psB.tile([128, 500], mybir.dt.float32,
                            name="gps00_0", tag="gps00_0")
            for _ in range(16 if "warm" not in skip else 1):
                nc.tensor.matmul(warm[:, 0:252], wsrc[:, 0:128],
                                 wsrc[:, 0:252], start=True, stop=True)
            # consts for iter 1+: delay so they don't steal early DMA slots
            with tc.tile_wait_until(0.006):
                nc.scalar.dma_start(wt_t[:, :], wt2[:, :])
                nc.scalar.dma_start(dlT[:, :], dlt[:, :])
            load_p1(2)

            # ---- steady state: the L2 W-fold chain leads each iteration
            # (its split feeds this iteration's last matmuls) ----
            for p in range(PAIRS + 1):
                if p >= 1:
                    wfold2(p - 1)         # needs h1[p-1] (prev iter)
                    split2(p - 1)
                if p < PAIRS:
                    atype_epi(p, 0)
                if p >= 1:
                    atype_epi(p - 1, 1)
                if p + 3 < PAIRS:
                    load_p1(p + 3)

    nc.compile()
    return nc


def _host_prep(x, edge_index, W1, b1, W2, b2):
    x = np.ascontiguousarray(np.asarray(x, dtype=np.float32))
    ei = np.asarray(edge_index)
    row, col = ei[0], ei[1]
    deg = np.zeros(N, np.float32)
    np.add.at(deg, col, 1.0)
    deg += 1.0
    dinv = (1.0 / np.sqrt(deg)).astype(np.float32)
    Mc = np.zeros((N, N), np.float32)
    np.add.at(Mc, (col, row), 1.0)
    Mc[np.arange(N), np.arange(N)] += 1.0
    MT = np.ascontiguousarray(Mc.T)                     # [src, dst]
    sd = (1.0 / dinv).astype(np.float32)                # sqrt(deg)

    # M^T packed into 8 chunks of 126 partitions, dst axis plain 1000.
    # Partition 125 of chunk 0 is the rank-1 bias rhs row (sqrt(deg)).
    pk = np.zeros((128, 8, N), np.float32)
    for k in range(8):
        pk[0:125, k, :] = MT[125 * k:125 * k + 125]
    pk[125, 0, :] = sd
    mth = np.ascontiguousarray(pk.astype(F8).reshape(128, 8 * N))

    W1_16 = np.asarray(W1).astype(np.float16)
    W2_16 = np.asarray(W2).astype(np.float16)
    wt2 = np.zeros((128, 128), np.float16)
    wt2[:64, :64] = W2_16
    wt2[64:, 64:] = W2_16
    u2 = np.linalg.solve(W2_16.astype(np.float64).T,
                         LAM * np.asarray(b2).astype(np.float64))
    uct = np.zeros((128, 8), np.float16)
    uct[:, 0] = np.tile(u2.astype(np.float16), 2)

    d2t = np.ascontiguousarray(np.broadcast_to(
        (dinv * dinv).astype(np.float16), (128, N)).copy())
    dlt = np.ascontiguousarray(np.broadcast_to(
        (dinv / LAM).astype(np.float16), (128, N)).copy())

    # Layer-1 P tiles: P1' = (lam*dinv (.) X)(fp16) @ W1(fp16), computed on
    # the host (an input-side linear fold), packed into the augmented
    # [q(126-part), k(8), (2b,f)] layout with the exact lam*b1 bias row,
    # and pre-split into fp8 hi/lo.
    scale = (LAM * dinv).astype(np.float32)
    bias_row = np.tile((LAM * np.asarray(b1)).astype(np.float32), 2)
    p1hs, p1ls = [], []
    for k in range(NCORES):
        c, hf = k // 2, k % 2
        slab = x[500 * hf:500 * hf + 500,
                 128 * c:128 * (c + 1), :].reshape(64000, D)
        xs = (slab.reshape(PAIRS, 2, N, D)
              * scale[None, None, :, None]).astype(np.float16)
        P = xs.astype(np.float32) @ W1_16.astype(np.float32)  # [P,2,N,64]
        # [pair, k, q, (2b,f)]
        Pa = np.zeros((PAIRS, 8, 126, 128), np.float32)
        Pa[:, :, 0:125, 0:64] = P[:, 0].reshape(PAIRS, 8, 125, 64)
        Pa[:, :, 0:125, 64:128] = P[:, 1].reshape(PAIRS, 8, 125, 64)
        Pa[:, 0, 125, :] = bias_row
        Pa = Pa.transpose(0, 2, 1, 3)                   # [pair, q, k, bf]
        hi = Pa.astype(F8)
        lo = (Pa - hi.astype(np.float32)).astype(F8)
        z = np.zeros((PAIRS, 2, 8, 128), F8)            # partitions 126/127
        p1hs.append(np.ascontiguousarray(np.concatenate(
            [hi, z], axis=1).reshape(PAIRS, 128, 8 * 128)))
        p1ls.append(np.ascontiguousarray(np.concatenate(
            [lo, z], axis=1).reshape(PAIRS, 128, 8 * 128)))
    return mth, wt2, uct, d2t, dlt, p1hs, p1ls


def kernel(x, edge_index, W1, b1, W2, b2):
    global _prog, LAST_RESULTS
    if _prog is None:
        _prog = _build_program()
    nc = _prog

    mth, wt2, uct, d2t, dlt, p1hs, p1ls = \
        _host_prep(x, edge_index, W1, b1, W2, b2)
    in_maps = [{"p1h": p1hs[k], "p1l": p1ls[k], "mth": mth, "wt2": wt2,
                "d2t": d2t, "dlt": dlt, "uct": uct}
               for k in range(NCORES)]

    LAST_RESULTS = run_bass_kernel_spmd(nc, in_maps,
                                        core_ids=list(range(NCORES)))

    out = np.empty((N, T, D), np.float32)
    for k in range(NCORES):
        c, hf = k // 2, k % 2
        ot = LAST_RESULTS.results[k]["xout"]          # [PAIRS, 128, N] f16
        # [pair, (2b, f), row] -> rows [pair*2+b]*1000+row, feat f
        slab = ot.reshape(PAIRS, 2, D, N).transpose(0, 1, 3, 2) \
                 .reshape(64000, D).astype(np.float32)
        out[500 * hf:500 * hf + 500, 128 * c:128 * (c + 1), :] = \
            slab.reshape(500, CS, D)
    return out
